# revision 20
# baseline (speedup 1.0000x reference)
"""Trainium2 Bass kernel for nn_Attention (soft spatial attention + fc).

Math (per batch b):
    z1 = hidden[0,b] @ W1.T + b1                       (ATT,)
    t  = tanh(att_img_features[b] + z1)                (AREA, ATT)
    z2 = t @ W2[0]  (+ b2 -- dropped: softmax-invariant)  (AREA,)
    a  = softmax(z2)                                   (AREA,)
    self = a @ img[b];  cap = a_cap[b] @ img[b]        (C,) each
    emb  = relu(concat(self, cap) @ W3.T + b3)         (C,)
Outputs: (emb (B,C), a (B,AREA,1))

Sharding: pure data parallel over batch, 8 cores x 32 batches.  Weights
replicated.  Host pre-transposes W1/W3/att/hidden/a_cap so the device
kernel only streams natural-layout tiles.
"""

import os
import sys

import numpy as np

for _p in ("/opt/trn_rl_repo", "/root/.axon_site/_ro/trn_rl_repo"):
    if os.path.isdir(_p) and _p not in sys.path:
        sys.path.insert(0, _p)

import concourse.bass as bass
import concourse.tile as tile
from concourse import bacc, mybir
from concourse.bass_utils import run_bass_kernel_spmd

F32 = mybir.dt.float32
F32R = mybir.dt.float32r

B, AREA, C, ATT, RNN = 256, 196, 2048, 512, 512
NCORES = 8
BL = B // NCORES          # 32 local batches per core
NG = BL // 2              # 16 groups of 2 for fc2
SCH = [(0, 98), (98, 98)]   # AREA=196 split into two 98-row s-chunks
KA = ATT // 128           # 4 a-chunks
KR = RNN // 128           # 4 r-chunks
KC = C // 512             # 4 c-tiles of 512 (psum bank width)
C2 = 2 * C


def _mm(nc, out, lhsT, rhs, start, stop):
    nc.tensor.matmul(out, lhsT, rhs, start=start, stop=stop)


def build_kernel():
    nc = bacc.Bacc(trn_type="TRN2", target_bir_lowering=False, debug=False)

    img_d = nc.dram_tensor("img", [BL, AREA, C], mybir.dt.bfloat16, kind="ExternalInput").ap()
    attT_d = nc.dram_tensor("attT", [BL, ATT, AREA], F32, kind="ExternalInput").ap()
    hidT_d = nc.dram_tensor("hidT", [RNN, BL], F32, kind="ExternalInput").ap()
    acapT_d = nc.dram_tensor("acapT", [AREA, BL], F32, kind="ExternalInput").ap()
    w1T_d = nc.dram_tensor("w1T", [RNN, ATT], F32, kind="ExternalInput").ap()
    b1_d = nc.dram_tensor("b1row", [1, ATT], F32, kind="ExternalInput").ap()
    w2_d = nc.dram_tensor("w2colT", [128, KA], F32, kind="ExternalInput").ap()
    w3T_d = nc.dram_tensor("w3T", [C2, C], mybir.dt.bfloat16, kind="ExternalInput").ap()
    b3_d = nc.dram_tensor("b3row", [1, C], F32, kind="ExternalInput").ap()
    id32_d = nc.dram_tensor("id32", [32, 32], F32, kind="ExternalInput").ap()
    zeros_d = nc.dram_tensor("zeros", [98, 16 * 32], F32, kind="ExternalInput").ap()
    ones_d = nc.dram_tensor("onesrow", [1, BL], F32, kind="ExternalInput").ap()
    warm_d = nc.dram_tensor("warm", [128, 512], mybir.dt.bfloat16,
                            kind="ExternalInput").ap()

    emb_d = nc.dram_tensor("emb", [BL, C], F32, kind="ExternalOutput").ap()
    aout_d = nc.dram_tensor("aout", [BL, AREA], F32, kind="ExternalOutput").ap()

    with tile.TileContext(nc) as tc:
        _body(nc, tc, img_d, attT_d, hidT_d, acapT_d, w1T_d, b1_d, w2_d,
              w3T_d, b3_d, id32_d, zeros_d, ones_d, warm_d, emb_d, aout_d)
    nc.compile()
    return nc


def _body(nc, tc, img_d, attT_d, hidT_d, acapT_d, w1T_d, b1_d, w2_d,
          w3T_d, b3_d, id32_d, zeros_d, ones_d, warm_d, emb_d, aout_d):
    from contextlib import ExitStack
    BF16 = mybir.dt.bfloat16
    NG2 = 8  # fc2 groups per 16-batch half
    ctx = ExitStack()
    const = ctx.enter_context(tc.tile_pool(name="const", bufs=1))
    attp = ctx.enter_context(tc.tile_pool(name="attp", bufs=3))
    ttp = ctx.enter_context(tc.tile_pool(name="ttp", bufs=2))
    imgp = ctx.enter_context(tc.tile_pool(name="imgp", bufs=7))
    w3p = ctx.enter_context(tc.tile_pool(name="w3p", bufs=3))
    stagep = ctx.enter_context(tc.tile_pool(name="stagep", bufs=3))
    # PSUM: 4 banks big accumulators (shared slot), 3x1 bank small, 1 bank tr
    ps_big = ctx.enter_context(tc.tile_pool(name="ps_big", bufs=1, space="PSUM"))
    ps_small = ctx.enter_context(tc.tile_pool(name="ps_small", bufs=3,
                                              space="PSUM"))
    ps_tr = ctx.enter_context(tc.tile_pool(name="ps_tr", bufs=1, space="PSUM"))

    # ---- constants -------------------------------------------------------
    id32 = const.tile([32, 32], F32)
    nc.sync.dma_start(out=id32, in_=id32_d)
    ones1 = const.tile([1, BL], F32)
    nc.sync.dma_start(out=ones1, in_=ones_d)
    id32b = const.tile([32, 32], BF16)
    nc.gpsimd.dma_start(out=id32b, in_=id32_d)
    warmsb = const.tile([128, 512], BF16)
    nc.sync.dma_start(out=warmsb, in_=warm_d)

    w1sb = []
    for kr in range(KR):
        t = const.tile([128, ATT], F32, tag=f"w1_{kr}")
        nc.gpsimd.dma_start(out=t, in_=w1T_d[kr * 128:(kr + 1) * 128, :])
        w1sb.append(t)
    hidsb = []
    for kr in range(KR):
        t = const.tile([128, BL], F32, tag=f"hid_{kr}")
        nc.gpsimd.dma_start(out=t, in_=hidT_d[kr * 128:(kr + 1) * 128, :])
        hidsb.append(t)
    b1sb = const.tile([1, ATT], F32)
    nc.sync.dma_start(out=b1sb, in_=b1_d)
    w2sb = const.tile([128, KA], F32)
    nc.sync.dma_start(out=w2sb, in_=w2_d)
    b3sb = const.tile([1, C], BF16)
    nc.gpsimd.dma_start(out=b3sb, in_=b3_d)
    ones_r = const.tile([1, BL], BF16)
    nc.gpsimd.dma_start(out=ones_r, in_=ones_d)
    # acap columns, s split by parity (s = 2p + h)
    acapsb = []
    for h in range(2):
        t = const.tile([98, BL], BF16, tag=f"acap_{h}")
        nc.gpsimd.dma_start(out=t, in_=acapT_d[h::2, :])
        acapsb.append(t)

    # masked block-diagonal stationaries: block b' cols 32b'..32b'+31,
    # col 34b' = a column, col 34b'+1 = a_cap column of local batch b'
    a2m = {}
    for g16 in range(2):
        for h in range(2):
            t = const.tile([98, 16 * 32], BF16, tag=f"a2m_{g16}_{h}")
            nc.gpsimd.dma_start(out=t, in_=zeros_d)
            a2m[(g16, h)] = t

    # ---- phase 1: z1 = hid @ W1.T + b1 -> transpose to columns -----------
    z1p = ps_small.tile([BL, ATT], F32, tag="small")
    for kr in range(KR):
        _mm(nc, z1p, hidsb[kr], w1sb[kr], start=(kr == 0), stop=False)
    _mm(nc, z1p, ones1, b1sb, start=False, stop=True)
    z1sb = const.tile([BL, ATT], F32)
    nc.vector.tensor_copy(z1sb, z1p)

    zb = const.tile([128, KA, BL], F32)   # z1 columns: [a_in_chunk, ka, b]
    for ka in range(KA):
        trp = ps_small.tile([128, BL], F32, tag="small")
        nc.tensor.transpose(trp, z1sb[:, ka * 128:(ka + 1) * 128], id32)
        nc.vector.tensor_copy(zb[:, ka, :], trp)

    ct = const.tile([128, 8, 2, 2, BL], BF16)  # [p, ck, h, g16, m]
    s5s = []
    for g16 in range(2):
        # ---- phase 2+3 for this 16-batch half ----------------------------
        z2sb = const.tile([NG2, 2, AREA], F32, tag=f"z2sb_{g16}")
        for g2 in range(NG2):
            g = NG2 * g16 + g2
            tt = ttp.tile([128, KA, 2, AREA], F32, tag="tt")
            att = attp.tile([128, 2, KA, AREA], F32, tag="att")
            nc.gpsimd.dma_start(
                out=att,
                in_=attT_d[2 * g:2 * g + 2]
                    .rearrange("b (ka p) s -> p b ka s", p=128))
            for pair in range(2):
                b = 2 * g + pair
                for ka in range(KA):
                    nc.scalar.activation(
                        tt[:, ka, pair, :], att[:, pair, ka, :],
                        mybir.ActivationFunctionType.Tanh,
                        bias=zb[:, ka, b:b + 1])
            z2p = ps_small.tile([1, 2 * AREA], F32, tag="small")
            for ka in range(KA):
                _mm(nc, z2p, w2sb[:, ka:ka + 1], tt[:, ka, :, :],
                    start=(ka == 0), stop=(ka == KA - 1))
            zst = stagep.tile([1, 2 * AREA], F32, tag="zst")
            nc.scalar.copy(zst, z2p)
            nc.sync.dma_start(out=z2sb[g2:g2 + 1, :, :], in_=zst)

        if g16 == 0:
            # HAM warm burst: fp32r/LDW-heavy phases don't register as PE
            # activity; a sustained real-matmul burst releases the clock gate.
            wt = ps_tr.tile([1, 512], F32, tag="tr")
            for _ in range(14):
                nc.tensor.matmul(wt, warmsb[:, 0:1], warmsb,
                                 start=True, stop=True)

        # ---- phase 4: softmax over s, batched over the half --------------
        mx = const.tile([NG2, 2], F32, tag=f"mx_{g16}")
        nc.vector.reduce_max(mx, z2sb, axis=mybir.AxisListType.X, negate=True)
        asb = const.tile([NG2, 2, AREA], F32, tag=f"asb_{g16}")
        esum = const.tile([NG2, 2], F32, tag=f"es_{g16}")
        for pair in range(2):
            nc.scalar.activation(
                asb[:, pair, :], z2sb[:, pair, :],
                mybir.ActivationFunctionType.Exp,
                bias=mx[:, pair:pair + 1],
                accum_out=esum[:, pair:pair + 1])
        rcp = const.tile([NG2, 2], F32, tag=f"rc_{g16}")
        nc.vector.reciprocal(rcp, esum)
        for pair in range(2):
            nc.vector.tensor_scalar_mul(asb[:, pair, :], asb[:, pair, :],
                                        rcp[:, pair:pair + 1])
        nc.sync.dma_start(out=aout_d[16 * g16:16 * (g16 + 1), :], in_=asb)

        # ---- phase 4b: transpose a into the masked stationaries ----------
        for pair in range(2):
            for h in range(2):
                trp = ps_tr.tile([128, BL], F32, tag="tr")
                nc.tensor.transpose(trp[:98, :NG2], asb[:, pair, h::2],
                                    id32[:NG2, :NG2])
                dst = a2m[(g16, h)][:, 34 * pair::68]
                nc.vector.tensor_copy(dst, trp[:98, :NG2])
        for h in range(2):
            dst = a2m[(g16, h)][:, 1::34]
            nc.vector.tensor_copy(dst, acapsb[h][:, 16 * g16:16 * g16 + 16])

        # ---- phase 5: (self|cap) rows for 16 batches, accumulated --------
        # wait-absorber: pulls the DVE clock forward on PE so the matmuls
        # below carry a single sync wait each.
        ab = ps_tr.tile([1, BL], BF16, tag="tr")
        nc.tensor.transpose(ab[:1, :32], a2m[(g16, 0)][0:32, 0:1],
                            id32b[:32, :32])
        p5 = ps_big.tile([BL, C], F32, tag="big")
        for bp in range(16):
            b = 16 * g16 + bp
            im = imgp.tile([98, 2, C], BF16, tag="img")
            nc.gpsimd.dma_start(
                out=im, in_=img_d[b].rearrange("(p h) c -> p h c", h=2))
            for h in range(2):
                for kc in range(KC):
                    _mm(nc, p5[:, kc * 512:(kc + 1) * 512],
                        a2m[(g16, h)][:, 32 * bp:32 * bp + 32],
                        im[:, h, kc * 512:(kc + 1) * 512],
                        start=(bp == 0 and h == 0),
                        stop=(bp == 15 and h == 1))
        s5 = const.tile([BL, C], F32, tag=f"s5_{g16}")
        nc.vector.tensor_copy(s5, p5)
        s5s.append(s5)
        for ck in range(8):
            for h in range(2):
                ctp = ps_small.tile([128, BL], F32, tag="small")
                nc.tensor.transpose(
                    ctp[:, :32],
                    s5[:, ck * 256 + h:(ck + 1) * 256:2], id32)
                nc.vector.tensor_copy(ct[:, ck, h, g16, :], ctp[:, :32])

    # ---- phase 6: emb = relu(concat @ W3.T + b3) -------------------------
    ab = ps_tr.tile([1, BL], BF16, tag="tr")
    nc.tensor.transpose(ab[:1, :32], ct[0:32, 0, 0, 0, 0:1],
                        id32b[:32, :32])
    fp = ps_big.tile([BL, C], F32, tag="big")
    for kk in range(16):
        w3sb = w3p.tile([128, 2, C], BF16, tag="w3")
        nc.sync.dma_start(
            out=w3sb,
            in_=w3T_d[kk * 256:(kk + 1) * 256, :]
                .rearrange("(p h) c -> p h c", h=2))
        for h in range(2):
            chunk = kk * 2 + h
            j, ck = divmod(kk, 8)
            for ti in range(KC):
                _mm(nc, fp[:, ti * 512:(ti + 1) * 512],
                    ct[:, ck, h, :, j::2],
                    w3sb[:, h, ti * 512:(ti + 1) * 512],
                    start=(chunk == 0), stop=False)
    for ti in range(KC):
        _mm(nc, fp[:, ti * 512:(ti + 1) * 512], ones_r,
            b3sb[0:1, ti * 512:(ti + 1) * 512], start=False, stop=True)
    embsb = const.tile([BL, C], F32)
    nc.vector.tensor_scalar_max(embsb, fp, 0.0)
    nc.sync.dma_start(out=emb_d, in_=embsb)

    ctx.close()


_NC = None


def _get_nc():
    global _NC
    if _NC is None:
        _NC = build_kernel()
    return _NC


def make_in_maps(img_features, att_img_features, hidden, a_cap,
                 W1, b1, W2, b2, W3, b3):
    import ml_dtypes
    img = np.ascontiguousarray(
        np.asarray(img_features, dtype=np.float32).astype(ml_dtypes.bfloat16))
    att = np.asarray(att_img_features, dtype=np.float32)
    hid = np.asarray(hidden, dtype=np.float32)
    acap = np.asarray(a_cap, dtype=np.float32)
    w1T = np.ascontiguousarray(np.asarray(W1, np.float32).T)          # (RNN, ATT)
    b1r = np.asarray(b1, np.float32).reshape(1, ATT).copy()
    w2c = np.ascontiguousarray(
        np.asarray(W2, np.float32).reshape(KA, 128).T)                # (128, KA)
    w3T = np.ascontiguousarray(
        np.asarray(W3, np.float32).T.astype(ml_dtypes.bfloat16))      # (2C, C)
    b3r = np.asarray(b3, np.float32).reshape(1, C).copy()
    id32 = np.eye(32, dtype=np.float32)
    zeros = np.zeros((98, 16 * 32), dtype=np.float32)
    onesrow = np.ones((1, BL), dtype=np.float32)
    warm = np.ones((128, 512), dtype=ml_dtypes.bfloat16)
    # b2 dropped: softmax(z + const) == softmax(z)

    in_maps = []
    for r in range(NCORES):
        sl = slice(r * BL, (r + 1) * BL)
        in_maps.append({
            "img": img[sl],
            "attT": np.ascontiguousarray(att[sl].transpose(0, 2, 1)),
            "hidT": np.ascontiguousarray(hid[0, sl].T),
            "acapT": np.ascontiguousarray(acap[sl].T),
            "w1T": w1T, "b1row": b1r, "w2colT": w2c,
            "w3T": w3T, "b3row": b3r, "id32": id32, "zeros": zeros,
            "onesrow": onesrow, "warm": warm,
        })
    return in_maps


def kernel(img_features, att_img_features, hidden, a_cap,
           W1, b1, W2, b2, W3, b3, _trace=False):
    nc = _get_nc()
    in_maps = make_in_maps(img_features, att_img_features, hidden, a_cap,
                           W1, b1, W2, b2, W3, b3)
    res = run_bass_kernel_spmd(nc, in_maps, list(range(NCORES)), trace=_trace)
    outs = res.results
    emb = np.concatenate([outs[r]["emb"] for r in range(NCORES)], axis=0)
    a = np.concatenate([outs[r]["aout"] for r in range(NCORES)], axis=0)
    out = (emb, a.reshape(B, AREA, 1))
    if _trace:
        return out, res
    return out


# revision 21
# speedup vs baseline: 1.1182x; 1.1182x over previous
"""Trainium2 Bass kernel for nn_Attention (soft spatial attention + fc).

Math (per batch b):
    z1 = hidden[0,b] @ W1.T + b1                       (ATT,)
    t  = tanh(att_img_features[b] + z1)                (AREA, ATT)
    z2 = t @ W2[0]  (+ b2 -- dropped: softmax-invariant)  (AREA,)
    a  = softmax(z2)                                   (AREA,)
    self = a @ img[b];  cap = a_cap[b] @ img[b]        (C,) each
    emb  = relu(concat(self, cap) @ W3.T + b3)         (C,)
Outputs: (emb (B,C), a (B,AREA,1))

Sharding: pure data parallel over batch, 8 cores x 32 batches.  Weights
replicated.  Host pre-transposes W1/W3/att/hidden/a_cap so the device
kernel only streams natural-layout tiles.
"""

import os
import sys

import numpy as np

for _p in ("/opt/trn_rl_repo", "/root/.axon_site/_ro/trn_rl_repo"):
    if os.path.isdir(_p) and _p not in sys.path:
        sys.path.insert(0, _p)

import concourse.bass as bass
import concourse.tile as tile
from concourse import bacc, mybir
from concourse.bass_utils import run_bass_kernel_spmd

F32 = mybir.dt.float32
F32R = mybir.dt.float32r

B, AREA, C, ATT, RNN = 256, 196, 2048, 512, 512
NCORES = 8
BL = B // NCORES          # 32 local batches per core
NG = BL // 2              # 16 groups of 2 for fc2
SCH = [(0, 98), (98, 98)]   # AREA=196 split into two 98-row s-chunks
KA = ATT // 128           # 4 a-chunks
KR = RNN // 128           # 4 r-chunks
KC = C // 512             # 4 c-tiles of 512 (psum bank width)
C2 = 2 * C


def _mm(nc, out, lhsT, rhs, start, stop):
    nc.tensor.matmul(out, lhsT, rhs, start=start, stop=stop)


def build_kernel():
    nc = bacc.Bacc(trn_type="TRN2", target_bir_lowering=False, debug=False)

    img_d = nc.dram_tensor("img", [BL, AREA, C], mybir.dt.bfloat16, kind="ExternalInput").ap()
    attT_d = nc.dram_tensor("attT", [BL, ATT, AREA], F32, kind="ExternalInput").ap()
    hidT_d = nc.dram_tensor("hidT", [RNN, BL], F32, kind="ExternalInput").ap()
    acapT_d = nc.dram_tensor("acapT", [AREA, BL], F32, kind="ExternalInput").ap()
    w1T_d = nc.dram_tensor("w1T", [RNN, ATT], F32, kind="ExternalInput").ap()
    b1_d = nc.dram_tensor("b1row", [1, ATT], F32, kind="ExternalInput").ap()
    w2_d = nc.dram_tensor("w2colT", [128, KA], F32, kind="ExternalInput").ap()
    w3T_d = nc.dram_tensor("w3T", [C2, C], mybir.dt.bfloat16, kind="ExternalInput").ap()
    b3_d = nc.dram_tensor("b3row", [1, C], F32, kind="ExternalInput").ap()
    id32_d = nc.dram_tensor("id32", [32, 32], F32, kind="ExternalInput").ap()
    zeros_d = nc.dram_tensor("zeros", [98, 16 * 32], F32, kind="ExternalInput").ap()
    ones_d = nc.dram_tensor("onesrow", [1, BL], F32, kind="ExternalInput").ap()
    warm_d = nc.dram_tensor("warm", [128, 512], mybir.dt.bfloat16,
                            kind="ExternalInput").ap()

    emb_d = nc.dram_tensor("emb", [BL, C], F32, kind="ExternalOutput").ap()
    aout_d = nc.dram_tensor("aout", [BL, AREA], F32, kind="ExternalOutput").ap()

    with tile.TileContext(nc) as tc:
        _body(nc, tc, img_d, attT_d, hidT_d, acapT_d, w1T_d, b1_d, w2_d,
              w3T_d, b3_d, id32_d, zeros_d, ones_d, warm_d, emb_d, aout_d)
    nc.compile()
    return nc


def _body(nc, tc, img_d, attT_d, hidT_d, acapT_d, w1T_d, b1_d, w2_d,
          w3T_d, b3_d, id32_d, zeros_d, ones_d, warm_d, emb_d, aout_d):
    from contextlib import ExitStack
    BF16 = mybir.dt.bfloat16
    NG2 = 8  # fc2 groups per 16-batch half
    ctx = ExitStack()
    const = ctx.enter_context(tc.tile_pool(name="const", bufs=1))
    attp = ctx.enter_context(tc.tile_pool(name="attp", bufs=3))
    ttp = ctx.enter_context(tc.tile_pool(name="ttp", bufs=2))
    imgp = ctx.enter_context(tc.tile_pool(name="imgp", bufs=7))
    w3p = ctx.enter_context(tc.tile_pool(name="w3p", bufs=3))
    stagep = ctx.enter_context(tc.tile_pool(name="stagep", bufs=3))
    # PSUM: 4 banks big accumulators (shared slot), 3x1 bank small, 1 bank tr
    ps_big = ctx.enter_context(tc.tile_pool(name="ps_big", bufs=1, space="PSUM"))
    ps_small = ctx.enter_context(tc.tile_pool(name="ps_small", bufs=3,
                                              space="PSUM"))
    ps_tr = ctx.enter_context(tc.tile_pool(name="ps_tr", bufs=1, space="PSUM"))

    # ---- constants -------------------------------------------------------
    id32 = const.tile([32, 32], F32)
    nc.sync.dma_start(out=id32, in_=id32_d)
    ones1 = const.tile([1, BL], F32)
    nc.sync.dma_start(out=ones1, in_=ones_d)
    id32b = const.tile([32, 32], BF16)
    nc.gpsimd.dma_start(out=id32b, in_=id32_d)
    warmsb = const.tile([128, 512], BF16)
    nc.sync.dma_start(out=warmsb, in_=warm_d)

    w1sb = []
    for kr in range(KR):
        t = const.tile([128, ATT], F32, tag=f"w1_{kr}")
        nc.gpsimd.dma_start(out=t, in_=w1T_d[kr * 128:(kr + 1) * 128, :])
        w1sb.append(t)
    hidsb = []
    for kr in range(KR):
        t = const.tile([128, BL], F32, tag=f"hid_{kr}")
        nc.gpsimd.dma_start(out=t, in_=hidT_d[kr * 128:(kr + 1) * 128, :])
        hidsb.append(t)
    b1sb = const.tile([1, ATT], F32)
    nc.sync.dma_start(out=b1sb, in_=b1_d)
    w2sb = const.tile([128, KA], F32)
    nc.sync.dma_start(out=w2sb, in_=w2_d)
    b3sb = const.tile([1, C], BF16)
    nc.gpsimd.dma_start(out=b3sb, in_=b3_d)
    ones_r = const.tile([1, BL], BF16)
    nc.gpsimd.dma_start(out=ones_r, in_=ones_d)
    # acap columns, s split by parity (s = 2p + h)
    acapsb = []
    for h in range(2):
        t = const.tile([98, BL], BF16, tag=f"acap_{h}")
        nc.gpsimd.dma_start(out=t, in_=acapT_d[h::2, :])
        acapsb.append(t)

    # masked block-diagonal stationaries: block b' cols 32b'..32b'+31,
    # col 34b' = a column, col 34b'+1 = a_cap column of local batch b'
    a2m = {}
    for g16 in range(2):
        for h in range(2):
            t = const.tile([98, 16 * 32], BF16, tag=f"a2m_{g16}_{h}")
            nc.gpsimd.dma_start(out=t, in_=zeros_d)
            a2m[(g16, h)] = t

    # ---- phase 1: z1 = hid @ W1.T + b1 -> transpose to columns -----------
    z1p = ps_small.tile([BL, ATT], F32, tag="small")
    for kr in range(KR):
        _mm(nc, z1p, hidsb[kr], w1sb[kr], start=(kr == 0), stop=False)
    _mm(nc, z1p, ones1, b1sb, start=False, stop=True)
    z1sb = const.tile([BL, ATT], F32)
    nc.vector.tensor_copy(z1sb, z1p)

    zb = const.tile([128, KA, BL], F32)   # z1 columns: [a_in_chunk, ka, b]
    for ka in range(KA):
        trp = ps_small.tile([128, BL], F32, tag="small")
        nc.tensor.transpose(trp, z1sb[:, ka * 128:(ka + 1) * 128], id32)
        nc.vector.tensor_copy(zb[:, ka, :], trp)

    ct = const.tile([128, 8, 2, 2, BL], BF16)  # [p, ck, h, g16, m]
    s5s = []
    for g16 in range(2):
        # ---- phase 2+3 for this 16-batch half ----------------------------
        z2sb = const.tile([NG2, 2, AREA], F32, tag=f"z2sb_{g16}")
        for g2 in range(NG2):
            g = NG2 * g16 + g2
            tt = ttp.tile([128, KA, 2, AREA], F32, tag="tt")
            att = attp.tile([128, 2, KA, AREA], F32, tag="att")
            nc.gpsimd.dma_start(
                out=att,
                in_=attT_d[2 * g:2 * g + 2]
                    .rearrange("b (ka p) s -> p b ka s", p=128))
            for pair in range(2):
                b = 2 * g + pair
                for ka in range(KA):
                    nc.scalar.activation(
                        tt[:, ka, pair, :], att[:, pair, ka, :],
                        mybir.ActivationFunctionType.Tanh,
                        bias=zb[:, ka, b:b + 1])
            z2p = ps_small.tile([1, 2 * AREA], F32, tag="small")
            for ka in range(KA):
                _mm(nc, z2p, w2sb[:, ka:ka + 1], tt[:, ka, :, :],
                    start=(ka == 0), stop=(ka == KA - 1))
            zst = stagep.tile([1, 2 * AREA], F32, tag="zst")
            nc.scalar.copy(zst, z2p)
            nc.sync.dma_start(out=z2sb[g2:g2 + 1, :, :], in_=zst)

        if g16 == 0:
            # HAM warm burst: fp32r/LDW-heavy phases don't register as PE
            # activity; a sustained real-matmul burst releases the clock gate.
            wt = ps_tr.tile([1, 512], F32, tag="tr")
            for _ in range(14):
                nc.tensor.matmul(wt, warmsb[:, 0:1], warmsb,
                                 start=True, stop=True)

        # ---- phase 4: softmax over s, batched over the half --------------
        mx = const.tile([NG2, 2], F32, tag=f"mx_{g16}")
        nc.vector.reduce_max(mx, z2sb, axis=mybir.AxisListType.X, negate=True)
        asb = const.tile([NG2, 2, AREA], F32, tag=f"asb_{g16}")
        esum = const.tile([NG2, 2], F32, tag=f"es_{g16}")
        for pair in range(2):
            nc.scalar.activation(
                asb[:, pair, :], z2sb[:, pair, :],
                mybir.ActivationFunctionType.Exp,
                bias=mx[:, pair:pair + 1],
                accum_out=esum[:, pair:pair + 1])
        rcp = const.tile([NG2, 2], F32, tag=f"rc_{g16}")
        nc.vector.reciprocal(rcp, esum)
        for pair in range(2):
            nc.vector.tensor_scalar_mul(asb[:, pair, :], asb[:, pair, :],
                                        rcp[:, pair:pair + 1])
        nc.sync.dma_start(out=aout_d[16 * g16:16 * (g16 + 1), :], in_=asb)

        # ---- phase 4b: transpose a into the masked stationaries ----------
        for pair in range(2):
            for h in range(2):
                trp = ps_tr.tile([128, BL], F32, tag="tr")
                nc.tensor.transpose(trp[:98, :NG2], asb[:, pair, h::2],
                                    id32[:NG2, :NG2])
                dst = a2m[(g16, h)][:, 34 * pair::68]
                nc.vector.tensor_copy(dst, trp[:98, :NG2])
        for h in range(2):
            dst = a2m[(g16, h)][:, 1::34]
            nc.vector.tensor_copy(dst, acapsb[h][:, 16 * g16:16 * g16 + 16])

        # ---- phase 5: (self|cap) rows for 16 batches, accumulated --------
        # wait-absorber: pulls the DVE clock forward on PE so the matmuls
        # below carry a single sync wait each.
        ab = ps_tr.tile([1, BL], BF16, tag="tr")
        nc.tensor.transpose(ab[:1, :32], a2m[(g16, 0)][0:32, 0:1],
                            id32b[:32, :32])
        p5 = ps_big.tile([BL, C], F32, tag="big")
        for bp in range(16):
            b = 16 * g16 + bp
            im = imgp.tile([98, 2, C], BF16, tag="img")
            nc.gpsimd.dma_start(
                out=im, in_=img_d[b].rearrange("(p h) c -> p h c", h=2))
            for h in range(2):
                for kc in range(KC):
                    _mm(nc, p5[:, kc * 512:(kc + 1) * 512],
                        a2m[(g16, h)][:, 32 * bp:32 * bp + 32],
                        im[:, h, kc * 512:(kc + 1) * 512],
                        start=(bp == 0 and h == 0),
                        stop=(bp == 15 and h == 1))
        s5 = const.tile([BL, C], F32, tag=f"s5_{g16}")
        nc.vector.tensor_copy(s5, p5)
        s5s.append(s5)
        for ck in range(8):
            for h in range(2):
                ctp = ps_small.tile([128, BL], F32, tag="small")
                nc.tensor.transpose(
                    ctp[:, :32],
                    s5[:, ck * 256 + h:(ck + 1) * 256:2], id32)
                nc.vector.tensor_copy(ct[:, ck, h, g16, :], ctp[:, :32])

    # ---- phase 6: emb = relu(concat @ W3.T + b3) -------------------------
    ab = ps_tr.tile([1, BL], BF16, tag="tr")
    nc.tensor.transpose(ab[:1, :32], ct[0:32, 0, 0, 0, 0:1],
                        id32b[:32, :32])
    fp = ps_big.tile([BL, C], F32, tag="big")
    for kk in range(16):
        w3sb = w3p.tile([128, 2, C], BF16, tag="w3")
        nc.gpsimd.dma_start(
            out=w3sb,
            in_=w3T_d[kk * 256:(kk + 1) * 256, :]
                .rearrange("(p h) c -> p h c", h=2))
        for h in range(2):
            chunk = kk * 2 + h
            j, ck = divmod(kk, 8)
            for ti in range(KC):
                _mm(nc, fp[:, ti * 512:(ti + 1) * 512],
                    ct[:, ck, h, :, j::2],
                    w3sb[:, h, ti * 512:(ti + 1) * 512],
                    start=(chunk == 0), stop=False)
    for ti in range(KC):
        _mm(nc, fp[:, ti * 512:(ti + 1) * 512], ones_r,
            b3sb[0:1, ti * 512:(ti + 1) * 512], start=False, stop=True)
    embsb = const.tile([BL, C], F32)
    nc.vector.tensor_scalar_max(embsb, fp, 0.0)
    nc.sync.dma_start(out=emb_d, in_=embsb)

    ctx.close()


_NC = None


def _get_nc():
    global _NC
    if _NC is None:
        _NC = build_kernel()
    return _NC


def make_in_maps(img_features, att_img_features, hidden, a_cap,
                 W1, b1, W2, b2, W3, b3):
    import ml_dtypes
    img = np.ascontiguousarray(
        np.asarray(img_features, dtype=np.float32).astype(ml_dtypes.bfloat16))
    att = np.asarray(att_img_features, dtype=np.float32)
    hid = np.asarray(hidden, dtype=np.float32)
    acap = np.asarray(a_cap, dtype=np.float32)
    w1T = np.ascontiguousarray(np.asarray(W1, np.float32).T)          # (RNN, ATT)
    b1r = np.asarray(b1, np.float32).reshape(1, ATT).copy()
    w2c = np.ascontiguousarray(
        np.asarray(W2, np.float32).reshape(KA, 128).T)                # (128, KA)
    w3T = np.ascontiguousarray(
        np.asarray(W3, np.float32).T.astype(ml_dtypes.bfloat16))      # (2C, C)
    b3r = np.asarray(b3, np.float32).reshape(1, C).copy()
    id32 = np.eye(32, dtype=np.float32)
    zeros = np.zeros((98, 16 * 32), dtype=np.float32)
    onesrow = np.ones((1, BL), dtype=np.float32)
    warm = np.ones((128, 512), dtype=ml_dtypes.bfloat16)
    # b2 dropped: softmax(z + const) == softmax(z)

    in_maps = []
    for r in range(NCORES):
        sl = slice(r * BL, (r + 1) * BL)
        in_maps.append({
            "img": img[sl],
            "attT": np.ascontiguousarray(att[sl].transpose(0, 2, 1)),
            "hidT": np.ascontiguousarray(hid[0, sl].T),
            "acapT": np.ascontiguousarray(acap[sl].T),
            "w1T": w1T, "b1row": b1r, "w2colT": w2c,
            "w3T": w3T, "b3row": b3r, "id32": id32, "zeros": zeros,
            "onesrow": onesrow, "warm": warm,
        })
    return in_maps


def kernel(img_features, att_img_features, hidden, a_cap,
           W1, b1, W2, b2, W3, b3, _trace=False):
    nc = _get_nc()
    in_maps = make_in_maps(img_features, att_img_features, hidden, a_cap,
                           W1, b1, W2, b2, W3, b3)
    res = run_bass_kernel_spmd(nc, in_maps, list(range(NCORES)), trace=_trace)
    outs = res.results
    emb = np.concatenate([outs[r]["emb"] for r in range(NCORES)], axis=0)
    a = np.concatenate([outs[r]["aout"] for r in range(NCORES)], axis=0)
    out = (emb, a.reshape(B, AREA, 1))
    if _trace:
        return out, res
    return out


# revision 22
# speedup vs baseline: 1.1256x; 1.0066x over previous
"""Trainium2 Bass kernel for nn_Attention (soft spatial attention + fc).

Math (per batch b):
    z1 = hidden[0,b] @ W1.T + b1                       (ATT,)
    t  = tanh(att_img_features[b] + z1)                (AREA, ATT)
    z2 = t @ W2[0]  (+ b2 -- dropped: softmax-invariant)  (AREA,)
    a  = softmax(z2)                                   (AREA,)
    self = a @ img[b];  cap = a_cap[b] @ img[b]        (C,) each
    emb  = relu(concat(self, cap) @ W3.T + b3)         (C,)
Outputs: (emb (B,C), a (B,AREA,1))

Sharding: pure data parallel over batch, 8 cores x 32 batches.  Weights
replicated.  Host pre-transposes W1/W3/att/hidden/a_cap so the device
kernel only streams natural-layout tiles.
"""

import os
import sys

import numpy as np

for _p in ("/opt/trn_rl_repo", "/root/.axon_site/_ro/trn_rl_repo"):
    if os.path.isdir(_p) and _p not in sys.path:
        sys.path.insert(0, _p)

import concourse.bass as bass
import concourse.tile as tile
from concourse import bacc, mybir
from concourse.bass_utils import run_bass_kernel_spmd

F32 = mybir.dt.float32
F32R = mybir.dt.float32r

B, AREA, C, ATT, RNN = 256, 196, 2048, 512, 512
NCORES = 8
BL = B // NCORES          # 32 local batches per core
NG = BL // 2              # 16 groups of 2 for fc2
SCH = [(0, 98), (98, 98)]   # AREA=196 split into two 98-row s-chunks
KA = ATT // 128           # 4 a-chunks
KR = RNN // 128           # 4 r-chunks
KC = C // 512             # 4 c-tiles of 512 (psum bank width)
C2 = 2 * C


def _mm(nc, out, lhsT, rhs, start, stop):
    nc.tensor.matmul(out, lhsT, rhs, start=start, stop=stop)


def build_kernel():
    nc = bacc.Bacc(trn_type="TRN2", target_bir_lowering=False, debug=False)

    img_d = nc.dram_tensor("img", [BL, AREA, C], mybir.dt.bfloat16, kind="ExternalInput").ap()
    attT_d = nc.dram_tensor("attT", [BL, ATT, AREA], F32, kind="ExternalInput").ap()
    hidT_d = nc.dram_tensor("hidT", [RNN, BL], F32, kind="ExternalInput").ap()
    acapT_d = nc.dram_tensor("acapT", [AREA, BL], F32, kind="ExternalInput").ap()
    w1T_d = nc.dram_tensor("w1T", [RNN, ATT], F32, kind="ExternalInput").ap()
    b1_d = nc.dram_tensor("b1row", [1, ATT], F32, kind="ExternalInput").ap()
    w2_d = nc.dram_tensor("w2colT", [128, KA], F32, kind="ExternalInput").ap()
    w3T_d = nc.dram_tensor("w3T", [C2, C], mybir.dt.bfloat16, kind="ExternalInput").ap()
    b3_d = nc.dram_tensor("b3row", [1, C], F32, kind="ExternalInput").ap()
    id32_d = nc.dram_tensor("id32", [32, 32], F32, kind="ExternalInput").ap()
    zeros_d = nc.dram_tensor("zeros", [98, 16 * 32], F32, kind="ExternalInput").ap()
    ones_d = nc.dram_tensor("onesrow", [1, BL], F32, kind="ExternalInput").ap()
    warm_d = nc.dram_tensor("warm", [128, 512], mybir.dt.bfloat16,
                            kind="ExternalInput").ap()

    emb_d = nc.dram_tensor("emb", [BL, C], F32, kind="ExternalOutput").ap()
    aout_d = nc.dram_tensor("aout", [BL, AREA], F32, kind="ExternalOutput").ap()

    with tile.TileContext(nc) as tc:
        _body(nc, tc, img_d, attT_d, hidT_d, acapT_d, w1T_d, b1_d, w2_d,
              w3T_d, b3_d, id32_d, zeros_d, ones_d, warm_d, emb_d, aout_d)
    nc.compile()
    return nc


def _body(nc, tc, img_d, attT_d, hidT_d, acapT_d, w1T_d, b1_d, w2_d,
          w3T_d, b3_d, id32_d, zeros_d, ones_d, warm_d, emb_d, aout_d):
    from contextlib import ExitStack
    BF16 = mybir.dt.bfloat16
    NG2 = 8  # fc2 groups per 16-batch half
    ctx = ExitStack()
    const = ctx.enter_context(tc.tile_pool(name="const", bufs=1))
    attp = ctx.enter_context(tc.tile_pool(name="attp", bufs=3))
    ttp = ctx.enter_context(tc.tile_pool(name="ttp", bufs=2))
    imgp = ctx.enter_context(tc.tile_pool(name="imgp", bufs=7))
    w3p = ctx.enter_context(tc.tile_pool(name="w3p", bufs=3))
    stagep = ctx.enter_context(tc.tile_pool(name="stagep", bufs=3))
    # PSUM: 4 banks big accumulators (shared slot), 3x1 bank small, 1 bank tr
    ps_big = ctx.enter_context(tc.tile_pool(name="ps_big", bufs=1, space="PSUM"))
    ps_small = ctx.enter_context(tc.tile_pool(name="ps_small", bufs=3,
                                              space="PSUM"))
    ps_tr = ctx.enter_context(tc.tile_pool(name="ps_tr", bufs=1, space="PSUM"))

    # ---- constants -------------------------------------------------------
    id32 = const.tile([32, 32], F32)
    nc.sync.dma_start(out=id32, in_=id32_d)
    ones1 = const.tile([1, BL], F32)
    nc.sync.dma_start(out=ones1, in_=ones_d)
    id32b = const.tile([32, 32], BF16)
    nc.gpsimd.dma_start(out=id32b, in_=id32_d)
    warmsb = const.tile([128, 512], BF16)
    nc.sync.dma_start(out=warmsb, in_=warm_d)

    w1sb = []
    for kr in range(KR):
        t = const.tile([128, ATT], F32, tag=f"w1_{kr}")
        nc.gpsimd.dma_start(out=t, in_=w1T_d[kr * 128:(kr + 1) * 128, :])
        w1sb.append(t)
    hidsb = []
    for kr in range(KR):
        t = const.tile([128, BL], F32, tag=f"hid_{kr}")
        nc.gpsimd.dma_start(out=t, in_=hidT_d[kr * 128:(kr + 1) * 128, :])
        hidsb.append(t)
    b1sb = const.tile([1, ATT], F32)
    nc.sync.dma_start(out=b1sb, in_=b1_d)
    w2sb = const.tile([128, KA], mybir.dt.float16)
    nc.gpsimd.dma_start(out=w2sb, in_=w2_d)
    b3sb = const.tile([1, C], BF16)
    nc.gpsimd.dma_start(out=b3sb, in_=b3_d)
    ones_r = const.tile([1, BL], BF16)
    nc.gpsimd.dma_start(out=ones_r, in_=ones_d)
    # acap columns, s split by parity (s = 2p + h)
    acapsb = []
    for h in range(2):
        t = const.tile([98, BL], BF16, tag=f"acap_{h}")
        nc.gpsimd.dma_start(out=t, in_=acapT_d[h::2, :])
        acapsb.append(t)

    # masked block-diagonal stationaries: block b' cols 32b'..32b'+31,
    # col 34b' = a column, col 34b'+1 = a_cap column of local batch b'
    a2m = {}
    for g16 in range(2):
        for h in range(2):
            t = const.tile([98, 16 * 32], BF16, tag=f"a2m_{g16}_{h}")
            nc.gpsimd.dma_start(out=t, in_=zeros_d)
            a2m[(g16, h)] = t

    # ---- phase 1: z1 = hid @ W1.T + b1 -> transpose to columns -----------
    z1p = ps_small.tile([BL, ATT], F32, tag="small")
    for kr in range(KR):
        _mm(nc, z1p, hidsb[kr], w1sb[kr], start=(kr == 0), stop=False)
    _mm(nc, z1p, ones1, b1sb, start=False, stop=True)
    z1sb = const.tile([BL, ATT], F32)
    nc.vector.tensor_copy(z1sb, z1p)

    zb = const.tile([128, KA, BL], F32)   # z1 columns: [a_in_chunk, ka, b]
    for ka in range(KA):
        trp = ps_small.tile([128, BL], F32, tag="small")
        nc.tensor.transpose(trp, z1sb[:, ka * 128:(ka + 1) * 128], id32)
        nc.vector.tensor_copy(zb[:, ka, :], trp)

    ct = const.tile([128, 8, 2, 2, BL], BF16)  # [p, ck, h, g16, m]
    s5s = []
    for g16 in range(2):
        # ---- phase 2+3 for this 16-batch half ----------------------------
        z2sb = const.tile([NG2, 2, AREA], F32, tag=f"z2sb_{g16}")
        for g2 in range(NG2):
            g = NG2 * g16 + g2
            tt = ttp.tile([128, KA, 2, AREA], mybir.dt.float16, tag="tt")
            att = attp.tile([128, 2, KA, AREA], F32, tag="att")
            nc.gpsimd.dma_start(
                out=att,
                in_=attT_d[2 * g:2 * g + 2]
                    .rearrange("b (ka p) s -> p b ka s", p=128))
            for pair in range(2):
                b = 2 * g + pair
                for ka in range(KA):
                    nc.vector.tensor_scalar_add(
                        att[:, pair, ka, :], att[:, pair, ka, :],
                        zb[:, ka, b:b + 1])
                nc.scalar.activation(
                    tt.rearrange("p ka pr s -> p pr ka s")[:, pair, :, :],
                    att[:, pair, :, :],
                    mybir.ActivationFunctionType.Tanh)
            z2p = ps_small.tile([1, 2 * AREA], F32, tag="small")
            for ka in range(KA):
                _mm(nc, z2p, w2sb[:, ka:ka + 1], tt[:, ka, :, :],
                    start=(ka == 0), stop=(ka == KA - 1))
            zst = stagep.tile([1, 2 * AREA], F32, tag="zst")
            nc.scalar.copy(zst, z2p)
            nc.sync.dma_start(out=z2sb[g2:g2 + 1, :, :], in_=zst)

        if g16 == 0:
            # HAM warm burst: fp32r/LDW-heavy phases don't register as PE
            # activity; a sustained real-matmul burst releases the clock gate.
            wt = ps_tr.tile([1, 512], F32, tag="tr")
            for _ in range(14):
                nc.tensor.matmul(wt, warmsb[:, 0:1], warmsb,
                                 start=True, stop=True)

        # ---- phase 4: softmax over s, batched over the half --------------
        mx = const.tile([NG2, 2], F32, tag=f"mx_{g16}")
        nc.vector.reduce_max(mx, z2sb, axis=mybir.AxisListType.X, negate=True)
        asb = const.tile([NG2, 2, AREA], F32, tag=f"asb_{g16}")
        esum = const.tile([NG2, 2], F32, tag=f"es_{g16}")
        for pair in range(2):
            nc.scalar.activation(
                asb[:, pair, :], z2sb[:, pair, :],
                mybir.ActivationFunctionType.Exp,
                bias=mx[:, pair:pair + 1],
                accum_out=esum[:, pair:pair + 1])
        rcp = const.tile([NG2, 2], F32, tag=f"rc_{g16}")
        nc.vector.reciprocal(rcp, esum)
        for pair in range(2):
            nc.vector.tensor_scalar_mul(asb[:, pair, :], asb[:, pair, :],
                                        rcp[:, pair:pair + 1])
        nc.sync.dma_start(out=aout_d[16 * g16:16 * (g16 + 1), :], in_=asb)

        # ---- phase 4b: transpose a into the masked stationaries ----------
        for pair in range(2):
            for h in range(2):
                trp = ps_tr.tile([128, BL], F32, tag="tr")
                nc.tensor.transpose(trp[:98, :NG2], asb[:, pair, h::2],
                                    id32[:NG2, :NG2])
                dst = a2m[(g16, h)][:, 34 * pair::68]
                nc.vector.tensor_copy(dst, trp[:98, :NG2])
        for h in range(2):
            dst = a2m[(g16, h)][:, 1::34]
            nc.vector.tensor_copy(dst, acapsb[h][:, 16 * g16:16 * g16 + 16])

        # ---- phase 5: (self|cap) rows for 16 batches, accumulated --------
        # wait-absorber: pulls the DVE clock forward on PE so the matmuls
        # below carry a single sync wait each.
        ab = ps_tr.tile([1, BL], BF16, tag="tr")
        nc.tensor.transpose(ab[:1, :32], a2m[(g16, 0)][0:32, 0:1],
                            id32b[:32, :32])
        p5 = ps_big.tile([BL, C], F32, tag="big")
        for bp in range(16):
            b = 16 * g16 + bp
            im = imgp.tile([98, 2, C], BF16, tag="img")
            nc.gpsimd.dma_start(
                out=im, in_=img_d[b].rearrange("(p h) c -> p h c", h=2))
            for h in range(2):
                for kc in range(KC):
                    _mm(nc, p5[:, kc * 512:(kc + 1) * 512],
                        a2m[(g16, h)][:, 32 * bp:32 * bp + 32],
                        im[:, h, kc * 512:(kc + 1) * 512],
                        start=(bp == 0 and h == 0),
                        stop=(bp == 15 and h == 1))
        s5 = const.tile([BL, C], F32, tag=f"s5_{g16}")
        nc.vector.tensor_copy(s5, p5)
        s5s.append(s5)
        for ck in range(8):
            for h in range(2):
                ctp = ps_small.tile([128, BL], F32, tag="small")
                nc.tensor.transpose(
                    ctp[:, :32],
                    s5[:, ck * 256 + h:(ck + 1) * 256:2], id32)
                nc.vector.tensor_copy(ct[:, ck, h, g16, :], ctp[:, :32])

    # ---- phase 6: emb = relu(concat @ W3.T + b3) -------------------------
    ab = ps_tr.tile([1, BL], BF16, tag="tr")
    nc.tensor.transpose(ab[:1, :32], ct[0:32, 0, 0, 0, 0:1],
                        id32b[:32, :32])
    fp = ps_big.tile([BL, C], F32, tag="big")
    for kk in range(16):
        w3sb = w3p.tile([128, 2, C], BF16, tag="w3")
        nc.gpsimd.dma_start(
            out=w3sb,
            in_=w3T_d[kk * 256:(kk + 1) * 256, :]
                .rearrange("(p h) c -> p h c", h=2))
        for h in range(2):
            chunk = kk * 2 + h
            j, ck = divmod(kk, 8)
            for ti in range(KC):
                _mm(nc, fp[:, ti * 512:(ti + 1) * 512],
                    ct[:, ck, h, :, j::2],
                    w3sb[:, h, ti * 512:(ti + 1) * 512],
                    start=(chunk == 0), stop=False)
    for ti in range(KC):
        _mm(nc, fp[:, ti * 512:(ti + 1) * 512], ones_r,
            b3sb[0:1, ti * 512:(ti + 1) * 512], start=False, stop=True)
    embsb = const.tile([BL, C], F32)
    nc.vector.tensor_scalar_max(embsb, fp, 0.0)
    nc.sync.dma_start(out=emb_d, in_=embsb)

    ctx.close()


_NC = None


def _get_nc():
    global _NC
    if _NC is None:
        _NC = build_kernel()
    return _NC


def make_in_maps(img_features, att_img_features, hidden, a_cap,
                 W1, b1, W2, b2, W3, b3):
    import ml_dtypes
    img = np.ascontiguousarray(
        np.asarray(img_features, dtype=np.float32).astype(ml_dtypes.bfloat16))
    att = np.asarray(att_img_features, dtype=np.float32)
    hid = np.asarray(hidden, dtype=np.float32)
    acap = np.asarray(a_cap, dtype=np.float32)
    w1T = np.ascontiguousarray(np.asarray(W1, np.float32).T)          # (RNN, ATT)
    b1r = np.asarray(b1, np.float32).reshape(1, ATT).copy()
    w2c = np.ascontiguousarray(
        np.asarray(W2, np.float32).reshape(KA, 128).T)                # (128, KA)
    w3T = np.ascontiguousarray(
        np.asarray(W3, np.float32).T.astype(ml_dtypes.bfloat16))      # (2C, C)
    b3r = np.asarray(b3, np.float32).reshape(1, C).copy()
    id32 = np.eye(32, dtype=np.float32)
    zeros = np.zeros((98, 16 * 32), dtype=np.float32)
    onesrow = np.ones((1, BL), dtype=np.float32)
    warm = np.ones((128, 512), dtype=ml_dtypes.bfloat16)
    # b2 dropped: softmax(z + const) == softmax(z)

    in_maps = []
    for r in range(NCORES):
        sl = slice(r * BL, (r + 1) * BL)
        in_maps.append({
            "img": img[sl],
            "attT": np.ascontiguousarray(att[sl].transpose(0, 2, 1)),
            "hidT": np.ascontiguousarray(hid[0, sl].T),
            "acapT": np.ascontiguousarray(acap[sl].T),
            "w1T": w1T, "b1row": b1r, "w2colT": w2c,
            "w3T": w3T, "b3row": b3r, "id32": id32, "zeros": zeros,
            "onesrow": onesrow, "warm": warm,
        })
    return in_maps


def kernel(img_features, att_img_features, hidden, a_cap,
           W1, b1, W2, b2, W3, b3, _trace=False):
    nc = _get_nc()
    in_maps = make_in_maps(img_features, att_img_features, hidden, a_cap,
                           W1, b1, W2, b2, W3, b3)
    res = run_bass_kernel_spmd(nc, in_maps, list(range(NCORES)), trace=_trace)
    outs = res.results
    emb = np.concatenate([outs[r]["emb"] for r in range(NCORES)], axis=0)
    a = np.concatenate([outs[r]["aout"] for r in range(NCORES)], axis=0)
    out = (emb, a.reshape(B, AREA, 1))
    if _trace:
        return out, res
    return out
